# revision 17
# baseline (speedup 1.0000x reference)
"""Trainium2 Bass kernel for nn_DLCF_DCA (scatter_memory).

Reference, per sample b (B=128, S=256, H=768, K=64):
  keep_dep[s]  = (s==0) or any_k(depend[b,k] == s-1)
  keep_dpd[s]  = (s==0) or any_k(depended[b,k] == s-1)
  mult[s]      = 1 at s==0; 0 if s-1 in no_connect; else w2 if s-1 in
                 depended, else w1 if s-1 in depend, else 0
  y1 = x * keep_dep;  y2 = x * keep_dpd;  y3 = x * mult

All three outputs are mostly zero rows (~22-31% nonzero).  The runtime
donates pre-zeroed output buffers (bass2jax.run_bass_via_pjrt), so the
kernel only touches the nonzero rows, in bf16 (rel err ~2^-8 << 2e-2):

  per core (16 samples, x shard [4096, 768] bf16):
    dma_gather the nonzero row sets from HBM into SBUF (row j at
    partition j%128, slot j//128), scale y3 rows by a per-row scalar
    table on the vector engine, then dma_scatter_add onto the zeroed
    outputs.  Index tables are int16 [128, N/16] (j at [j%16, j//16],
    replicated across gpsimd cores) padded with trailing -1 which the
    ucode trims per core at runtime; the same table drives both the
    gather and the scatter (y[r] = x[r] * scale).

Q7 descriptor generation (~9 ns/row) is the dominant cost, so the 8
SWDGE instructions are spread over 4 SWDGE queues.  num_idxs per
instruction is capped at 1024, so y3 (~1300 rows) is split in two
position chunks; the chunks scatter into two separate full-size output
tensors (y3 = y3a + y3b on the host) so no write-after-write ordering
serializes them on device.
"""

import contextlib
import os
import sys

import numpy as np

if "/opt/trn_rl_repo" not in sys.path:
    sys.path.insert(0, "/opt/trn_rl_repo")

import ml_dtypes

N_CORES = 8
B, S, H, K = 128, 256, 768, 64
BL = B // N_CORES          # samples per core
ROWS = BL * S              # 4096 rows per core

MAXN = 1024  # dma_gather/scatter_add limit on num_idxs per instruction

_cache = {}


def _chunk_sizes(n):
    """Split n into even 16-aligned chunks of at most MAXN."""
    k = (n + MAXN - 1) // MAXN
    per = ((n + k - 1) // k + 15) // 16 * 16
    out, left = [], n
    for _ in range(k):
        out.append(min(per, max(16, (left + 15) // 16 * 16)))
        left -= out[-1]
    return out


def _slots(n):
    return (n + 127) // 128


def _build(n1, n2, n3):
    """n1/n2/n3: static num_idxs per stream (multiples of 16)."""
    import concourse.bacc as bacc
    import concourse.tile as tile
    from concourse import mybir

    f32 = mybir.dt.float32
    bf16 = mybir.dt.bfloat16
    i16 = mybir.dt.int16
    mul = mybir.AluOpType.mult

    chunks = [_chunk_sizes(n) for n in (n1, n2, n3)]
    cols = [sum(c // 16 for c in ch) for ch in chunks]
    sl3 = sum(_slots(c) for c in chunks[2])

    nc = bacc.Bacc(None, num_swdge_queues=4)
    x = nc.dram_tensor("x", [ROWS, H], bf16, kind="ExternalInput")
    widx = nc.dram_tensor("widx", [128, 8], i16, kind="ExternalInput")
    idxs = [nc.dram_tensor(f"idx{i + 1}", [128, cols[i]], i16,
                           kind="ExternalInput") for i in range(3)]
    scal = nc.dram_tensor("scal", [128, sl3], f32, kind="ExternalInput")
    # one output tensor per scatter instruction: no WAW ordering on device;
    # host sums the y3 chunk tensors (disjoint rows, zeros elsewhere).
    outs = {}
    for i in range(3):
        for ci in range(len(chunks[i])):
            outs[(i, ci)] = nc.dram_tensor(
                f"y{i + 1}{'abcd'[ci]}", [ROWS, H], bf16,
                kind="ExternalOutput")

    with tile.TileContext(nc) as tc, contextlib.ExitStack() as ctx:
        pool = ctx.enter_context(tc.tile_pool(name="pool", bufs=1))

        # tiny warmup gather: absorbs the Q7 library first-use init while the
        # real index tables are still loading (its idx DMA is issued first)
        wit = pool.tile([128, 8], i16, name="wit")
        nc.sync.dma_start(out=wit[:], in_=widx[:])
        wgt = pool.tile([128, 1, H], bf16, name="wgt")
        nc.gpsimd.dma_gather(wgt[:], x[:], wit[:], 128, 128, H, queue_num=0)

        its = []
        for i in range(3):
            it = pool.tile([128, cols[i]], i16, name=f"i{i + 1}")
            nc.sync.dma_start(out=it[:], in_=idxs[i][:])
            its.append(it)
        sc = pool.tile([128, sl3], f32, name="sc")
        nc.sync.dma_start(out=sc[:], in_=scal[:])

        # y3 gathers first: their scatters have the longest dep chain
        order = [(2, ci) for ci in range(len(chunks[2]))] + \
                [(0, ci) for ci in range(len(chunks[0]))] + \
                [(1, ci) for ci in range(len(chunks[1]))]
        colof = {}
        for i in range(3):
            c0 = 0
            for ci, n in enumerate(chunks[i]):
                colof[(i, ci)] = c0
                c0 += n // 16

        gts = {}
        q = 0
        for i, ci in order:
            n = chunks[i][ci]
            gt = pool.tile([128, _slots(n), H], bf16, name=f"g{i + 1}_{ci}")
            ix = its[i][:, colof[(i, ci)] : colof[(i, ci)] + n // 16]
            nc.gpsimd.dma_gather(gt[:], x[:], ix, n, n, H, queue_num=q % 4)
            gts[(i, ci)] = (gt, ix)
            q += 1

        # y3 rows scaled by per-row scalar (f32 table, bf16 data)
        ybs = {}
        s0 = 0
        for ci, n in enumerate(chunks[2]):
            gt, _ = gts[(2, ci)]
            yb = pool.tile([128, _slots(n), H], bf16, name=f"y3b_{ci}")
            for s in range(_slots(n)):
                nc.vector.tensor_scalar(
                    yb[:, s, :], gt[:, s, :], sc[:, s0 + s : s0 + s + 1],
                    None, op0=mul,
                )
            s0 += _slots(n)
            ybs[ci] = yb

        q = 0
        for i, ci in order:
            n = chunks[i][ci]
            gt, ix = gts[(i, ci)]
            src = ybs[ci] if i == 2 else gt
            nc.gpsimd.dma_scatter_add(outs[(i, ci)][:], src[:], ix, n, n, H,
                                      queue_num=q % 4)
            q += 1
    nc.finalize()
    return nc


def _wrap16(vals, n):
    """Index list -> int16 [128, n/16] table: j at [j%16, j//16], -1 padded,
    replicated across the 8 gpsimd cores."""
    t = np.full((16, n // 16), -1, dtype=np.int16)
    m = len(vals)
    j = np.arange(m)
    t[j % 16, j // 16] = vals
    return np.tile(t, (8, 1))


def _prep_inputs(bert_local_out, depend, depended, no_connect,
                 depend_weight, depended_weight):
    x = np.asarray(bert_local_out, dtype=np.float32).reshape(B, S, H)
    xb = x.astype(ml_dtypes.bfloat16)
    w1 = np.asarray(depend_weight, dtype=np.float32)
    w2 = np.asarray(depended_weight, dtype=np.float32)

    def row_sets(idx_arr):
        a = np.asarray(idx_arr, dtype=np.int64)
        out = []
        for b in range(B):
            v = a[b]
            v = v[(v >= 0) & (v <= S - 2)]
            out.append(np.unique(v) + 1)
        return out

    D = row_sets(depend)
    P = row_sets(depended)
    N = row_sets(no_connect)

    rows = [[None] * 3 for _ in range(N_CORES)]
    scal3 = [None] * N_CORES
    for c in range(N_CORES):
        r1l, r2l, r3l, s3l = [], [], [], []
        for bl in range(BL):
            b = c * BL + bl
            base = bl * S
            r1l.append(base + np.concatenate(([0], D[b])))
            r2l.append(base + np.concatenate(([0], P[b])))
            u = np.union1d(D[b], P[b])
            u = u[~np.isin(u, N[b])]
            sc = np.where(np.isin(u, P[b]), w2[b], w1[b])
            r3l.append(base + np.concatenate(([0], u)))
            s3l.append(np.concatenate(([1.0], sc)).astype(np.float32))
        rows[c][0] = np.concatenate(r1l)
        rows[c][1] = np.concatenate(r2l)
        rows[c][2] = np.concatenate(r3l)
        scal3[c] = np.concatenate(s3l)

    def rup16(v):
        return max(16, (v + 15) // 16 * 16)

    n1 = rup16(max(len(r[0]) for r in rows))
    n2 = rup16(max(len(r[1]) for r in rows))
    n3 = rup16(max(len(r[2]) for r in rows))

    def tables(rowlist, n):
        parts, at = [], 0
        for cn in _chunk_sizes(n):
            parts.append(_wrap16(rowlist[at : at + cn], cn))
            at += cn
        return np.concatenate(parts, axis=1)

    in_maps = []
    for c in range(N_CORES):
        m = {"x": np.ascontiguousarray(xb[c * BL : (c + 1) * BL]).reshape(ROWS, H),
             "widx": np.zeros((128, 8), dtype=np.int16)}
        for i in range(3):
            m[f"idx{i + 1}"] = tables(rows[c][i], (n1, n2, n3)[i])
        ch3 = _chunk_sizes(n3)
        sct = np.zeros((128, sum(_slots(cn) for cn in ch3)), dtype=np.float32)
        at = s0 = 0
        for cn in ch3:
            v = scal3[c][at : at + cn]
            j = np.arange(len(v))
            sct[j % 128, s0 + j // 128] = v
            at += cn
            s0 += _slots(cn)
        m["scal"] = sct
        in_maps.append(m)
    return in_maps, (n1, n2, n3)


def kernel(bert_local_out, depend, depended, no_connect,
           depend_weight, depended_weight):
    from concourse.bass_utils import run_bass_kernel_spmd

    in_maps, key = _prep_inputs(bert_local_out, depend, depended, no_connect,
                                depend_weight, depended_weight)
    if key not in _cache:
        _cache.clear()
        _cache[key] = _build(*key)
    nc = _cache[key]

    pdir = os.environ.get("KERNEL_PROFILE_DIR")
    ctx = contextlib.nullcontext()
    if pdir:
        import concourse.bass2jax as b2j
        from trn_agent_boot.trn_boot import _ntff_profile_via_ctypes

        if not getattr(b2j, "_neff_capture_patched", False):
            orig = b2j.rename_neff_tensors_and_patch_header

            def patched(neff_path, mapping):
                data = orig(neff_path, mapping)
                cap = os.environ.get("KERNEL_PROFILE_DIR")
                if cap:
                    os.makedirs(cap, exist_ok=True)
                    with open(os.path.join(cap, "model.neff"), "wb") as f:
                        f.write(data)
                return data

            b2j.rename_neff_tensors_and_patch_header = patched
            b2j._neff_capture_patched = True
        os.makedirs(pdir, exist_ok=True)
        hookf = _ntff_profile_via_ctypes("/opt/axon/libaxon_pjrt.so")
        if hookf is not None:
            dev = None if os.environ.get("KERNEL_PROFILE_ALL") else [0]
            ctx = hookf(pdir, dev)

    with ctx:
        res = run_bass_kernel_spmd(nc, in_maps, list(range(N_CORES)))

    nchunks = [len(_chunk_sizes(n)) for n in key]
    outs = []
    for i in range(3):
        full = np.empty((B, S, H), dtype=np.float32)
        for c in range(N_CORES):
            acc = np.asarray(res.results[c][f"y{i + 1}a"]).astype(np.float32)
            for ci in range(1, nchunks[i]):
                acc += np.asarray(
                    res.results[c][f"y{i + 1}{'abcd'[ci]}"]).astype(np.float32)
            full[c * BL : (c + 1) * BL] = acc.reshape(BL, S, H)
        outs.append(full)
    return tuple(outs)


# revision 18
# speedup vs baseline: 1.0442x; 1.0442x over previous
"""Trainium2 Bass kernel for nn_DLCF_DCA (scatter_memory).

Reference, per sample b (B=128, S=256, H=768, K=64):
  keep_dep[s]  = (s==0) or any_k(depend[b,k] == s-1)
  keep_dpd[s]  = (s==0) or any_k(depended[b,k] == s-1)
  mult[s]      = 1 at s==0; 0 if s-1 in no_connect; else w2 if s-1 in
                 depended, else w1 if s-1 in depend, else 0
  y1 = x * keep_dep;  y2 = x * keep_dpd;  y3 = x * mult

All three outputs are mostly zero rows (~22-31% nonzero).  The runtime
donates pre-zeroed output buffers (bass2jax.run_bass_via_pjrt), so the
kernel only touches the nonzero rows, in bf16 (rel err ~2^-8 << 2e-2):

  per core (16 samples, x shard [4096, 768] bf16):
    dma_gather the nonzero row sets from HBM into SBUF (row j at
    partition j%128, slot j//128), scale y3 rows by a per-row scalar
    table on the vector engine, then dma_scatter_add onto the zeroed
    outputs.  Index tables are int16 [128, N/16] (j at [j%16, j//16],
    replicated across gpsimd cores) padded with trailing -1 which the
    ucode trims per core at runtime; the same table drives both the
    gather and the scatter (y[r] = x[r] * scale).

Q7 descriptor generation (~9 ns/row) is the dominant cost, so the 8
SWDGE instructions are spread over 4 SWDGE queues.  num_idxs per
instruction is capped at 1024, so y3 (~1300 rows) is split in two
position chunks; the chunks scatter into two separate full-size output
tensors (y3 = y3a + y3b on the host) so no write-after-write ordering
serializes them on device.
"""

import contextlib
import os
import sys

import numpy as np

if "/opt/trn_rl_repo" not in sys.path:
    sys.path.insert(0, "/opt/trn_rl_repo")

import ml_dtypes

N_CORES = 8
B, S, H, K = 128, 256, 768, 64
BL = B // N_CORES          # samples per core
ROWS = BL * S              # 4096 rows per core

MAXN = 1024  # dma_gather/scatter_add limit on num_idxs per instruction

_cache = {}


def _chunk_sizes(n):
    """Split n into even 16-aligned chunks of at most MAXN."""
    k = (n + MAXN - 1) // MAXN
    per = ((n + k - 1) // k + 15) // 16 * 16
    out, left = [], n
    for _ in range(k):
        out.append(min(per, max(16, (left + 15) // 16 * 16)))
        left -= out[-1]
    return out


def _slots(n):
    return (n + 127) // 128


def _build(n1, n2, n3):
    """n1/n2/n3: static num_idxs per stream (multiples of 16)."""
    import concourse.bacc as bacc
    import concourse.tile as tile
    from concourse import mybir

    f32 = mybir.dt.float32
    bf16 = mybir.dt.bfloat16
    i16 = mybir.dt.int16
    mul = mybir.AluOpType.mult

    chunks = [_chunk_sizes(n) for n in (n1, n2, n3)]
    cols = [sum(c // 16 for c in ch) for ch in chunks]
    sl3 = sum(_slots(c) for c in chunks[2])

    nc = bacc.Bacc(None, num_swdge_queues=4)
    x = nc.dram_tensor("x", [ROWS, H], bf16, kind="ExternalInput")
    idxs = [nc.dram_tensor(f"idx{i + 1}", [128, cols[i]], i16,
                           kind="ExternalInput") for i in range(3)]
    scal = nc.dram_tensor("scal", [128, sl3], f32, kind="ExternalInput")
    # one output tensor per scatter instruction: no WAW ordering on device;
    # host sums the y3 chunk tensors (disjoint rows, zeros elsewhere).
    outs = {}
    for i in range(3):
        for ci in range(len(chunks[i])):
            outs[(i, ci)] = nc.dram_tensor(
                f"y{i + 1}{'abcd'[ci]}", [ROWS, H], bf16,
                kind="ExternalOutput")

    with tile.TileContext(nc) as tc, contextlib.ExitStack() as ctx:
        pool = ctx.enter_context(tc.tile_pool(name="pool", bufs=1))

        its = []
        for i in range(3):
            it = pool.tile([128, cols[i]], i16, name=f"i{i + 1}")
            nc.sync.dma_start(out=it[:], in_=idxs[i][:])
            its.append(it)
        sc = pool.tile([128, sl3], f32, name="sc")
        nc.sync.dma_start(out=sc[:], in_=scal[:])

        # y3 gathers first: their scatters have the longest dep chain
        order = [(2, ci) for ci in range(len(chunks[2]))] + \
                [(0, ci) for ci in range(len(chunks[0]))] + \
                [(1, ci) for ci in range(len(chunks[1]))]
        colof = {}
        for i in range(3):
            c0 = 0
            for ci, n in enumerate(chunks[i]):
                colof[(i, ci)] = c0
                c0 += n // 16

        gts = {}
        q = 0
        for i, ci in order:
            n = chunks[i][ci]
            gt = pool.tile([128, _slots(n), H], bf16, name=f"g{i + 1}_{ci}")
            ix = its[i][:, colof[(i, ci)] : colof[(i, ci)] + n // 16]
            nc.gpsimd.dma_gather(gt[:], x[:], ix, n, n, H, queue_num=q % 4)
            gts[(i, ci)] = (gt, ix)
            q += 1

        # y3 rows scaled by per-row scalar (f32 table, bf16 data)
        ybs = {}
        s0 = 0
        for ci, n in enumerate(chunks[2]):
            gt, _ = gts[(2, ci)]
            yb = pool.tile([128, _slots(n), H], bf16, name=f"y3b_{ci}")
            for s in range(_slots(n)):
                nc.vector.tensor_scalar(
                    yb[:, s, :], gt[:, s, :], sc[:, s0 + s : s0 + s + 1],
                    None, op0=mul,
                )
            s0 += _slots(n)
            ybs[ci] = yb

        q = 0
        for i, ci in order:
            n = chunks[i][ci]
            gt, ix = gts[(i, ci)]
            src = ybs[ci] if i == 2 else gt
            nc.gpsimd.dma_scatter_add(outs[(i, ci)][:], src[:], ix, n, n, H,
                                      queue_num=q % 4)
            q += 1
    nc.finalize()
    return nc


def _wrap16(vals, n):
    """Index list -> int16 [128, n/16] table: j at [j%16, j//16], -1 padded,
    replicated across the 8 gpsimd cores."""
    t = np.full((16, n // 16), -1, dtype=np.int16)
    m = len(vals)
    j = np.arange(m)
    t[j % 16, j // 16] = vals
    return np.tile(t, (8, 1))


def _prep_inputs(bert_local_out, depend, depended, no_connect,
                 depend_weight, depended_weight):
    x = np.asarray(bert_local_out, dtype=np.float32).reshape(B, S, H)
    xb = x.astype(ml_dtypes.bfloat16)
    w1 = np.asarray(depend_weight, dtype=np.float32)
    w2 = np.asarray(depended_weight, dtype=np.float32)

    def row_sets(idx_arr):
        a = np.asarray(idx_arr, dtype=np.int64)
        out = []
        for b in range(B):
            v = a[b]
            v = v[(v >= 0) & (v <= S - 2)]
            out.append(np.unique(v) + 1)
        return out

    D = row_sets(depend)
    P = row_sets(depended)
    N = row_sets(no_connect)

    rows = [[None] * 3 for _ in range(N_CORES)]
    scal3 = [None] * N_CORES
    for c in range(N_CORES):
        r1l, r2l, r3l, s3l = [], [], [], []
        for bl in range(BL):
            b = c * BL + bl
            base = bl * S
            r1l.append(base + np.concatenate(([0], D[b])))
            r2l.append(base + np.concatenate(([0], P[b])))
            u = np.union1d(D[b], P[b])
            u = u[~np.isin(u, N[b])]
            sc = np.where(np.isin(u, P[b]), w2[b], w1[b])
            r3l.append(base + np.concatenate(([0], u)))
            s3l.append(np.concatenate(([1.0], sc)).astype(np.float32))
        rows[c][0] = np.concatenate(r1l)
        rows[c][1] = np.concatenate(r2l)
        rows[c][2] = np.concatenate(r3l)
        scal3[c] = np.concatenate(s3l)

    def rup16(v):
        return max(16, (v + 15) // 16 * 16)

    n1 = rup16(max(len(r[0]) for r in rows))
    n2 = rup16(max(len(r[1]) for r in rows))
    n3 = rup16(max(len(r[2]) for r in rows))

    def tables(rowlist, n):
        parts, at = [], 0
        for cn in _chunk_sizes(n):
            parts.append(_wrap16(rowlist[at : at + cn], cn))
            at += cn
        return np.concatenate(parts, axis=1)

    in_maps = []
    for c in range(N_CORES):
        m = {"x": np.ascontiguousarray(xb[c * BL : (c + 1) * BL]).reshape(ROWS, H)}
        for i in range(3):
            m[f"idx{i + 1}"] = tables(rows[c][i], (n1, n2, n3)[i])
        ch3 = _chunk_sizes(n3)
        sct = np.zeros((128, sum(_slots(cn) for cn in ch3)), dtype=np.float32)
        at = s0 = 0
        for cn in ch3:
            v = scal3[c][at : at + cn]
            j = np.arange(len(v))
            sct[j % 128, s0 + j // 128] = v
            at += cn
            s0 += _slots(cn)
        m["scal"] = sct
        in_maps.append(m)
    return in_maps, (n1, n2, n3)


def kernel(bert_local_out, depend, depended, no_connect,
           depend_weight, depended_weight):
    from concourse.bass_utils import run_bass_kernel_spmd

    in_maps, key = _prep_inputs(bert_local_out, depend, depended, no_connect,
                                depend_weight, depended_weight)
    if key not in _cache:
        _cache.clear()
        _cache[key] = _build(*key)
    nc = _cache[key]

    pdir = os.environ.get("KERNEL_PROFILE_DIR")
    ctx = contextlib.nullcontext()
    if pdir:
        import concourse.bass2jax as b2j
        from trn_agent_boot.trn_boot import _ntff_profile_via_ctypes

        if not getattr(b2j, "_neff_capture_patched", False):
            orig = b2j.rename_neff_tensors_and_patch_header

            def patched(neff_path, mapping):
                data = orig(neff_path, mapping)
                cap = os.environ.get("KERNEL_PROFILE_DIR")
                if cap:
                    os.makedirs(cap, exist_ok=True)
                    with open(os.path.join(cap, "model.neff"), "wb") as f:
                        f.write(data)
                return data

            b2j.rename_neff_tensors_and_patch_header = patched
            b2j._neff_capture_patched = True
        os.makedirs(pdir, exist_ok=True)
        hookf = _ntff_profile_via_ctypes("/opt/axon/libaxon_pjrt.so")
        if hookf is not None:
            dev = None if os.environ.get("KERNEL_PROFILE_ALL") else [0]
            ctx = hookf(pdir, dev)

    with ctx:
        res = run_bass_kernel_spmd(nc, in_maps, list(range(N_CORES)))

    nchunks = [len(_chunk_sizes(n)) for n in key]
    outs = []
    for i in range(3):
        full = np.empty((B, S, H), dtype=np.float32)
        for c in range(N_CORES):
            acc = np.asarray(res.results[c][f"y{i + 1}a"]).astype(np.float32)
            for ci in range(1, nchunks[i]):
                acc += np.asarray(
                    res.results[c][f"y{i + 1}{'abcd'[ci]}"]).astype(np.float32)
            full[c * BL : (c + 1) * BL] = acc.reshape(BL, S, H)
        outs.append(full)
    return tuple(outs)


# revision 20
# speedup vs baseline: 1.0519x; 1.0074x over previous
"""Trainium2 Bass kernel for nn_DLCF_DCA (scatter_memory).

Reference, per sample b (B=128, S=256, H=768, K=64):
  keep_dep[s]  = (s==0) or any_k(depend[b,k] == s-1)
  keep_dpd[s]  = (s==0) or any_k(depended[b,k] == s-1)
  mult[s]      = 1 at s==0; 0 if s-1 in no_connect; else w2 if s-1 in
                 depended, else w1 if s-1 in depend, else 0
  y1 = x * keep_dep;  y2 = x * keep_dpd;  y3 = x * mult

All three outputs are mostly zero rows (~22-31% nonzero).  The runtime
donates pre-zeroed output buffers (bass2jax.run_bass_via_pjrt), so the
kernel only touches the nonzero rows, in bf16 (rel err ~2^-8 << 2e-2):

  per core (16 samples, x shard [4096, 768] bf16):
    dma_gather the nonzero row sets from HBM into SBUF (row j at
    partition j%128, slot j//128), scale y3 rows by a per-row scalar
    table on the vector engine, then dma_scatter_add onto the zeroed
    outputs.  Index tables are int16 [128, N/16] (j at [j%16, j//16],
    replicated across gpsimd cores) padded with trailing -1 which the
    ucode trims per core at runtime; the same table drives both the
    gather and the scatter (y[r] = x[r] * scale).

Q7 descriptor generation (~9 ns/row) is the dominant cost, so the 8
SWDGE instructions are spread over 4 SWDGE queues.  num_idxs per
instruction is capped at 1024, so y3 (~1300 rows) is split in two
position chunks; the chunks scatter into two separate full-size output
tensors (y3 = y3a + y3b on the host) so no write-after-write ordering
serializes them on device.
"""

import contextlib
import os
import sys

import numpy as np

if "/opt/trn_rl_repo" not in sys.path:
    sys.path.insert(0, "/opt/trn_rl_repo")

import ml_dtypes

N_CORES = 8
B, S, H, K = 128, 256, 768, 64
BL = B // N_CORES          # samples per core
ROWS = BL * S              # 4096 rows per core

MAXN = 1024  # dma_gather/scatter_add limit on num_idxs per instruction

_cache = {}


def _chunk_sizes(n):
    """Split n into even 16-aligned chunks of at most MAXN."""
    k = (n + MAXN - 1) // MAXN
    per = ((n + k - 1) // k + 15) // 16 * 16
    out, left = [], n
    for _ in range(k):
        out.append(min(per, max(16, (left + 15) // 16 * 16)))
        left -= out[-1]
    return out


def _slots(n):
    return (n + 127) // 128


def _build(n1, n2, n3):
    """n1/n2/n3: static num_idxs per stream (multiples of 16)."""
    import concourse.bacc as bacc
    import concourse.tile as tile
    from concourse import mybir

    f32 = mybir.dt.float32
    bf16 = mybir.dt.bfloat16
    i16 = mybir.dt.int16
    mul = mybir.AluOpType.mult

    chunks = [_chunk_sizes(n) for n in (n1, n2, n3)]
    cols = [sum(c // 16 for c in ch) for ch in chunks]
    sl3 = sum(_slots(c) for c in chunks[2])

    nc = bacc.Bacc(None, num_swdge_queues=4)
    x = nc.dram_tensor("x", [ROWS, H], bf16, kind="ExternalInput")
    idxs = [nc.dram_tensor(f"idx{i + 1}", [128, cols[i]], i16,
                           kind="ExternalInput") for i in range(3)]
    scal = nc.dram_tensor("scal", [128, sl3], f32, kind="ExternalInput")
    # one output tensor per scatter instruction: no WAW ordering on device;
    # host sums the y3 chunk tensors (disjoint rows, zeros elsewhere).
    outs = {}
    for i in range(3):
        for ci in range(len(chunks[i])):
            outs[(i, ci)] = nc.dram_tensor(
                f"y{i + 1}{'abcd'[ci]}", [ROWS, H], bf16,
                kind="ExternalOutput")

    with tile.TileContext(nc) as tc, contextlib.ExitStack() as ctx:
        pool = ctx.enter_context(tc.tile_pool(name="pool", bufs=1))

        its = []
        for i in range(3):
            it = pool.tile([128, cols[i]], i16, name=f"i{i + 1}")
            nc.sync.dma_start(out=it[:], in_=idxs[i][:])
            its.append(it)
        sc = pool.tile([128, sl3], f32, name="sc")
        nc.sync.dma_start(out=sc[:], in_=scal[:])

        # y3 gathers first: their scatters have the longest dep chain
        order = [(2, ci) for ci in range(len(chunks[2]))] + \
                [(0, ci) for ci in range(len(chunks[0]))] + \
                [(1, ci) for ci in range(len(chunks[1]))]
        colof = {}
        for i in range(3):
            c0 = 0
            for ci, n in enumerate(chunks[i]):
                colof[(i, ci)] = c0
                c0 += n // 16

        gts = {}
        q = 0
        for i, ci in order:
            n = chunks[i][ci]
            gt = pool.tile([128, _slots(n), H], bf16, name=f"g{i + 1}_{ci}")
            ix = its[i][:, colof[(i, ci)] : colof[(i, ci)] + n // 16]
            nc.gpsimd.dma_gather(gt[:], x[:], ix, n, n, H, queue_num=q % 4)
            gts[(i, ci)] = (gt, ix)
            q += 1

        # y3 rows scaled by per-row scalar (f32 table, bf16 data)
        ybs = {}
        s0 = 0
        for ci, n in enumerate(chunks[2]):
            gt, _ = gts[(2, ci)]
            yb = pool.tile([128, _slots(n), H], bf16, name=f"y3b_{ci}")
            for s in range(_slots(n)):
                nc.vector.tensor_scalar(
                    yb[:, s, :], gt[:, s, :], sc[:, s0 + s : s0 + s + 1],
                    None, op0=mul,
                )
            s0 += _slots(n)
            ybs[ci] = yb

        q = 0
        for i, ci in order:
            n = chunks[i][ci]
            gt, ix = gts[(i, ci)]
            src = ybs[ci] if i == 2 else gt
            nc.gpsimd.dma_scatter_add(outs[(i, ci)][:], src[:], ix, n, n, H,
                                      queue_num=q % 4)
            q += 1
    nc.finalize()
    return nc


def _wrap16(vals, n):
    """Index list -> int16 [128, n/16] table: j at [j%16, j//16], -1 padded,
    replicated across the 8 gpsimd cores."""
    t = np.full((16, n // 16), -1, dtype=np.int16)
    m = len(vals)
    j = np.arange(m)
    t[j % 16, j // 16] = vals
    return np.tile(t, (8, 1))


def _prep_inputs(bert_local_out, depend, depended, no_connect,
                 depend_weight, depended_weight):
    x = np.asarray(bert_local_out, dtype=np.float32).reshape(B, S, H)
    xb = x.astype(ml_dtypes.bfloat16)
    w1 = np.asarray(depend_weight, dtype=np.float32)
    w2 = np.asarray(depended_weight, dtype=np.float32)

    def row_sets(idx_arr):
        a = np.asarray(idx_arr, dtype=np.int64)
        out = []
        for b in range(B):
            v = a[b]
            v = v[(v >= 0) & (v <= S - 2)]
            out.append(np.unique(v) + 1)
        return out

    D = row_sets(depend)
    P = row_sets(depended)
    N = row_sets(no_connect)

    rows = [[None] * 3 for _ in range(N_CORES)]
    scal3 = [None] * N_CORES
    for c in range(N_CORES):
        r1l, r2l, r3l, s3l = [], [], [], []
        for bl in range(BL):
            b = c * BL + bl
            base = bl * S
            r1l.append(base + np.concatenate(([0], D[b])))
            r2l.append(base + np.concatenate(([0], P[b])))
            u = np.union1d(D[b], P[b])
            u = u[~np.isin(u, N[b])]
            sc = np.where(np.isin(u, P[b]), w2[b], w1[b])
            r3l.append(base + np.concatenate(([0], u)))
            s3l.append(np.concatenate(([1.0], sc)).astype(np.float32))
        rows[c][0] = np.concatenate(r1l)
        rows[c][1] = np.concatenate(r2l)
        rows[c][2] = np.concatenate(r3l)
        scal3[c] = np.concatenate(s3l)

    def rup16(v):
        return max(16, (v + 15) // 16 * 16)

    n1 = rup16(max(len(r[0]) for r in rows))
    n2 = rup16(max(len(r[1]) for r in rows))
    n3 = rup16(max(len(r[2]) for r in rows))

    def tables(rowlist, n):
        parts, at = [], 0
        for cn in _chunk_sizes(n):
            parts.append(_wrap16(rowlist[at : at + cn], cn))
            at += cn
        return np.concatenate(parts, axis=1)

    in_maps = []
    for c in range(N_CORES):
        m = {"x": np.ascontiguousarray(xb[c * BL : (c + 1) * BL]).reshape(ROWS, H)}
        for i in range(3):
            m[f"idx{i + 1}"] = tables(rows[c][i], (n1, n2, n3)[i])
        ch3 = _chunk_sizes(n3)
        sct = np.zeros((128, sum(_slots(cn) for cn in ch3)), dtype=np.float32)
        at = s0 = 0
        for cn in ch3:
            v = scal3[c][at : at + cn]
            j = np.arange(len(v))
            sct[j % 128, s0 + j // 128] = v
            at += cn
            s0 += _slots(cn)
        m["scal"] = sct
        in_maps.append(m)
    return in_maps, (n1, n2, n3)


def kernel(bert_local_out, depend, depended, no_connect,
           depend_weight, depended_weight):
    from concourse.bass_utils import run_bass_kernel_spmd

    in_maps, key = _prep_inputs(bert_local_out, depend, depended, no_connect,
                                depend_weight, depended_weight)
    if key not in _cache:
        _cache.clear()
        _cache[key] = _build(*key)
    nc = _cache[key]

    pdir = os.environ.get("KERNEL_PROFILE_DIR")
    ctx = contextlib.nullcontext()
    if pdir:
        import concourse.bass2jax as b2j
        from trn_agent_boot.trn_boot import _ntff_profile_via_ctypes

        if not getattr(b2j, "_neff_capture_patched", False):
            orig = b2j.rename_neff_tensors_and_patch_header

            def patched(neff_path, mapping):
                data = orig(neff_path, mapping)
                cap = os.environ.get("KERNEL_PROFILE_DIR")
                if cap:
                    os.makedirs(cap, exist_ok=True)
                    with open(os.path.join(cap, "model.neff"), "wb") as f:
                        f.write(data)
                return data

            b2j.rename_neff_tensors_and_patch_header = patched
            b2j._neff_capture_patched = True
        os.makedirs(pdir, exist_ok=True)
        hookf = _ntff_profile_via_ctypes("/opt/axon/libaxon_pjrt.so")
        if hookf is not None:
            dev = None if os.environ.get("KERNEL_PROFILE_ALL") else [0]
            ctx = hookf(pdir, dev)

    with ctx:
        res = run_bass_kernel_spmd(nc, in_maps, list(range(N_CORES)))

    nchunks = [len(_chunk_sizes(n)) for n in key]
    outs = []
    for i in range(3):
        full = np.empty((B, S, H), dtype=np.float32)
        for c in range(N_CORES):
            acc = np.asarray(res.results[c][f"y{i + 1}a"]).astype(np.float32)
            for ci in range(1, nchunks[i]):
                acc += np.asarray(
                    res.results[c][f"y{i + 1}{'abcd'[ci]}"]).astype(np.float32)
            full[c * BL : (c + 1) * BL] = acc.reshape(BL, S, H)
        outs.append(full)
    return tuple(outs)
